# revision 4
# baseline (speedup 1.0000x reference)
"""BitLinear (BitNet b1.58 ternary-weight linear) Trainium2 kernel.

Reference computation:
    scale = mean(|w|)                      # global scalar over the FULL weight
    w_q   = round(clip(w / (scale+1e-8), -1, 1)) * scale    # ternary {-1,0,1}*scale
    out   = einsum('bsi,oi->bso', x, w_q)  # x @ w_q.T

Sharding (8 NeuronCores, tensor-parallel on out_features):
    core c receives:
      xt  [4096, 4096] bf16  = x.reshape(4096,4096).T   (replicated; [d_in, tok])
      wt  [4096,  512] f32   = w.T[:, c*512:(c+1)*512]  ([d_in, d_out/8] shard)
    and produces:
      out [4096,  512] f32   = (x @ w_q.T)[:, c*512:(c+1)*512]

    The global scale needs the sum of |w| over ALL shards, so each core
    reduces its own shard and a 512-byte AllReduce combines the per-partition
    partials; a ones-matmul then reduces across partitions AND broadcasts the
    total to all 128 partitions in one shot.

Device pipeline per core:
  0. A dummy 4-byte AllReduce fires immediately at kernel start: the first
     collective pays a ~40us multi-rank entry-barrier; issuing it at t=0
     overlaps that cost with the weight-shard DMA + |w| reduction.
  1. DMA wt shard into SBUF (resident, prioritized over x prefetch),
     per-128-row k-tile reduce sum(|w|) -> per-partition partials.
  2. AllReduce partials across the 8 cores; ones-matmul -> broadcast total.
  3. thresh = 0.5*(scale+eps); ternary-quantize the shard to bf16 in 2 DVE
     ops per k-tile, producing the NEGATED ternary pattern
     (w < -thresh) - (w > thresh); the negation is undone by multiplying
     the output by -scale (both steps are exact, so no precision is lost).
  4. 1024 accumulating matmuls: stationary = x.T tile [128k x 128t] (bf16),
     moving = quantized w.T k-slab [128k x 512o], accumulated over the 32
     k-tiles into 8 PSUM banks (one per 128-token tile); evacuate each bank
     through the DVE with a fused multiply by -scale into fp32 staging.

Numerics: x is rounded to bf16 once (host side); everything else accumulates
in fp32 (PSUM) and the ternary weights are exact in bf16, so the end-to-end
error is ~1.7e-3 relative (bf16 input rounding), far inside the usual gates.
"""

import numpy as np
import ml_dtypes

import concourse.bass as bass
import concourse.bacc as bacc
import concourse.mybir as mybir
import concourse.tile as tile
from concourse.bass_utils import run_bass_kernel_spmd
from concourse.tile_rust import add_dep_helper

# Problem geometry (hardcoded per the contract).
B, S = 2, 2048
D_IN = 4096
D_OUT = 4096
N_CORES = 8

P = 128                      # SBUF/PSUM partitions
TOK = B * S                  # 4096 tokens
O_SHARD = D_OUT // N_CORES   # 512 output features per core
KT = D_IN // P               # 32 contraction k-tiles
TT = TOK // P                # 32 token tiles
NBANKS = 8                   # PSUM banks used as accumulators
NG = TT // NBANKS            # 4 token-tile groups
GCOLS = P * NBANKS           # 1024 tokens per group

F32 = mybir.dt.float32
BF16 = mybir.dt.bfloat16

EPS = np.float32(1e-8)
HALF_EPS = float(np.float32(0.5) * EPS)          # exact
NEG_INV_N = float(-np.float32(2.0 ** -24))       # -1/(4096*4096), exact
HALF_INV_N = float(np.float32(2.0 ** -25))


def _build_program():
    """Build and compile the per-core Bass program (identical on all cores)."""
    nc = bacc.Bacc("TRN2", target_bir_lowering=False, debug=False,
                   num_devices=N_CORES)

    xt = nc.dram_tensor("xt", [D_IN, TOK], BF16, kind="ExternalInput")
    wt = nc.dram_tensor("wt", [D_IN, O_SHARD], F32, kind="ExternalInput")
    out = nc.dram_tensor("out", [TOK, O_SHARD], F32, kind="ExternalOutput")

    rg = [list(range(N_CORES))]

    with tile.TileContext(nc) as tc:
        with (
            tc.tile_pool(name="const", bufs=1) as const,
            tc.tile_pool(name="wf", bufs=1) as wf,
            tc.tile_pool(name="wq", bufs=1) as wqp,
            tc.tile_pool(name="small", bufs=1) as small,
            tc.tile_pool(name="qtmp", bufs=4) as qtmp,
            tc.tile_pool(name="xp", bufs=8) as xp,
            tc.tile_pool(name="op", bufs=4) as op,
            tc.tile_pool(name="ps", bufs=8, space="PSUM") as ps,
            tc.tile_pool(name="dram", bufs=1, space="DRAM") as dram,
        ):
            # ---- phase 0: dummy collective to absorb the entry barrier ----
            warm_sb = small.tile([1, 1], F32)
            nc.vector.memset(warm_sb[:], 0.0)
            warm_in = dram.tile([1, 1], F32)
            warm_out = dram.tile([1, 1], F32)
            nc.sync.dma_start(warm_in[:], warm_sb[:1, :1])
            nc.gpsimd.collective_compute(
                "AllReduce", mybir.AluOpType.add, replica_groups=rg,
                ins=[warm_in.opt()], outs=[warm_out.opt()],
            )

            ones_sb = const.tile([P, P], F32)
            nc.vector.memset(ones_sb[:], 1.0)

            # ---- phase 1: local sum(|w|) over the shard --------------------
            wt_sb = wf.tile([P, KT, O_SHARD], F32)       # resident fp32 shard
            partials = small.tile([P, KT], F32)
            w_dmas = []
            for k in range(KT):
                w_dmas.append(
                    nc.sync.dma_start(wt_sb[:, k, :], wt[k * P:(k + 1) * P, :]))
                nc.vector.tensor_reduce(
                    partials[:, k:k + 1], wt_sb[:, k, :],
                    axis=mybir.AxisListType.X, op=mybir.AluOpType.add,
                    apply_absolute_value=True,
                )
            partial1 = small.tile([P, 1], F32)
            nc.vector.tensor_reduce(
                partial1[:, 0:1], partials[:, :],
                axis=mybir.AxisListType.X, op=mybir.AluOpType.add,
            )

            # ---- AllReduce the per-partition partials across the 8 cores ---
            ar_in = dram.tile([P, 1], F32)
            ar_out = dram.tile([P, 1], F32)
            nc.sync.dma_start(ar_in[:], partial1[:, 0:1])
            nc.gpsimd.collective_compute(
                "AllReduce", mybir.AluOpType.add, replica_groups=rg,
                ins=[ar_in.opt()], outs=[ar_out.opt()],
            )
            gpart = small.tile([P, 1], F32)
            nc.sync.dma_start(gpart[:, 0:1], ar_out[:])

            # reduce across partitions AND broadcast: ones[128,128].T @ gpart
            psB = ps.tile([P, 512], F32, tag="acc", name="ps_bcast")
            nc.tensor.matmul(psB[:, 0:1], ones_sb[:, :], gpart[:, 0:1],
                             start=True, stop=True)

            nscale_sb = small.tile([P, 1], F32)
            thresh_sb = small.tile([P, 1], F32)
            nthresh_sb = small.tile([P, 1], F32)
            # -scale = total * -2^-24 (exact); thresh = 0.5*(scale+eps)
            # computed as total*2^-25 + eps/2, bit-identical to the reference
            # (power-of-2 scaling commutes with fp32 rounding).
            nc.vector.tensor_scalar_mul(nscale_sb[:, 0:1], psB[:, 0:1], NEG_INV_N)
            nc.vector.tensor_scalar(
                thresh_sb[:, 0:1], psB[:, 0:1], HALF_INV_N, HALF_EPS,
                mybir.AluOpType.mult, mybir.AluOpType.add,
            )
            nc.vector.tensor_scalar_mul(nthresh_sb[:, 0:1], thresh_sb[:, 0:1], -1.0)

            # ---- ternary quantize shard -> bf16 NEGATED {-1, 0, +1} --------
            # wq = (w < -thresh) - (w > thresh) = -ternary(w); undone by -scale.
            wq_sb = wqp.tile([P, KT, O_SHARD], BF16)     # resident ternary shard
            for k in range(KT):
                pos = qtmp.tile([P, O_SHARD], BF16, tag="pos", name=f"pos_{k}")
                nc.vector.tensor_scalar(
                    pos[:], wt_sb[:, k, :], thresh_sb[:, 0:1], None,
                    mybir.AluOpType.is_gt,
                )
                nc.vector.scalar_tensor_tensor(
                    wq_sb[:, k, :], wt_sb[:, k, :], nthresh_sb[:, 0:1], pos[:],
                    mybir.AluOpType.is_lt, mybir.AluOpType.subtract,
                )

            # ---- main matmul: out[t, o] = sum_k xt[k, t] * wq[k, o] --------
            first_group = True
            for g in range(NG):
                psums = [ps.tile([P, 512], F32, tag="acc", name=f"acc_{g}_{t}")
                         for t in range(NBANKS)]
                for k in range(KT):
                    xt_t = xp.tile([P, GCOLS], BF16, tag="xt", name=f"xt_{g}_{k}")
                    xd = nc.sync.dma_start(
                        xt_t[:],
                        xt[k * P:(k + 1) * P, g * GCOLS:(g + 1) * GCOLS],
                    )
                    if first_group and k < 10:
                        # keep the first x prefetches out of the weight-shard
                        # DMA's way: the scale AllReduce (and everything after
                        # it) is serialized behind the w DMAs, while x has
                        # ~50us of slack before the matmuls start.
                        add_dep_helper(xd.ins, w_dmas[-1].ins, True,
                                       "prioritize w shard DMA")
                    for t in range(NBANKS):
                        nc.tensor.matmul(
                            psums[t][:, :O_SHARD],
                            xt_t[:, t * P:(t + 1) * P],
                            wq_sb[:, k, :],
                            start=(k == 0), stop=(k == KT - 1),
                        )
                first_group = False
                for t in range(NBANKS):
                    ot = op.tile([P, O_SHARD], F32, tag="ot", name=f"ot_{g}_{t}")
                    nc.vector.tensor_scalar_mul(
                        ot[:], psums[t][:, :O_SHARD], nscale_sb[:, 0:1])
                    row = (g * NBANKS + t) * P
                    nc.sync.dma_start(out[row:row + P, :], ot[:])

    nc.compile()
    return nc


_NC_CACHE = None


def _get_program():
    global _NC_CACHE
    if _NC_CACHE is None:
        _NC_CACHE = _build_program()
    return _NC_CACHE


def _make_in_maps(input: np.ndarray, weight: np.ndarray):
    x2d = np.ascontiguousarray(input.reshape(TOK, D_IN))
    xt_np = np.ascontiguousarray(x2d.T).astype(ml_dtypes.bfloat16)
    wT = np.ascontiguousarray(weight.T)          # [d_in, d_out] fp32
    in_maps = []
    for c in range(N_CORES):
        in_maps.append({
            "xt": xt_np,
            "wt": np.ascontiguousarray(wT[:, c * O_SHARD:(c + 1) * O_SHARD]),
        })
    return in_maps


def run_device(input: np.ndarray, weight: np.ndarray, **spmd_kwargs):
    """Run the sharded kernel; returns (full_output, BassKernelResults)."""
    nc = _get_program()
    in_maps = _make_in_maps(input, weight)
    res = run_bass_kernel_spmd(nc, in_maps, list(range(N_CORES)), **spmd_kwargs)
    shards = [res.results[c]["out"] for c in range(N_CORES)]
    full = np.concatenate(shards, axis=1).reshape(B, S, D_OUT)
    return np.ascontiguousarray(full.astype(np.float32)), res


def kernel(input: np.ndarray, weight: np.ndarray) -> np.ndarray:
    out, _ = run_device(input, weight)
    return out


# revision 9
# speedup vs baseline: 1.1234x; 1.1234x over previous
"""BitLinear (BitNet b1.58 ternary-weight linear) Trainium2 kernel.

Reference computation:
    scale = mean(|w|)                      # global scalar over the FULL weight
    w_q   = round(clip(w / (scale+1e-8), -1, 1)) * scale    # ternary {-1,0,1}*scale
    out   = einsum('bsi,oi->bso', x, w_q)  # x @ w_q.T

Sharding (8 NeuronCores, tensor-parallel on out_features):
    core c receives:
      xt  [4096, 4096] bf16  = x.reshape(4096,4096).T   (replicated; [d_in, tok])
      wt  [4096,  512] f32   = w.T[:, c*512:(c+1)*512]  ([d_in, d_out/8] shard)
    and produces:
      out [4096,  512] f32   = (x @ w_q.T)[:, c*512:(c+1)*512]

    The global scale needs the sum of |w| over ALL shards, so each core
    reduces its own shard and a 512-byte AllReduce combines the per-partition
    partials; a ones-matmul then reduces across partitions AND broadcasts the
    total to all 128 partitions in one shot.

Device pipeline per core:
  1. DMA wt shard into SBUF (resident, prioritized over x prefetch),
     per-128-row k-tile reduce sum(|w|) -> per-partition partials.
  2. AllReduce partials across the 8 cores; ones-matmul -> broadcast total.
  3. thresh = 0.5*(scale+eps); ternary-quantize the shard to bf16 in 2 DVE
     ops per k-tile, producing the NEGATED ternary pattern
     (w < -thresh) - (w > thresh); the negation is undone by multiplying
     the output by -scale (both steps are exact, so no precision is lost).
  4. 1024 accumulating matmuls: stationary = x.T tile [128k x 128t] (bf16),
     moving = quantized w.T k-slab [128k x 512o], accumulated over the 32
     k-tiles into PSUM banks (one per 128-token tile). Token tiles are
     processed in groups of 4 banks with the other 4 banks evacuating
     concurrently (ping-pong), so the PE never stalls on PSUM slots;
     evacuation is a DVE copy with a fused multiply by -scale.

Numerics: x is rounded to bf16 once (host side); everything else accumulates
in fp32 (PSUM) and the ternary weights are exact in bf16, so the end-to-end
error is ~1.7e-3 relative (bf16 input rounding), far inside the usual gates.
"""

import numpy as np
import ml_dtypes

import concourse.bass as bass
import concourse.bacc as bacc
import concourse.mybir as mybir
import concourse.tile as tile
from concourse.bass_utils import run_bass_kernel_spmd
from concourse.tile_rust import add_dep_helper

# Problem geometry (hardcoded per the contract).
B, S = 2, 2048
D_IN = 4096
D_OUT = 4096
N_CORES = 8

P = 128                      # SBUF/PSUM partitions
TOK = B * S                  # 4096 tokens
O_SHARD = D_OUT // N_CORES   # 512 output features per core
KT = D_IN // P               # 32 contraction k-tiles
TT = TOK // P                # 32 token tiles
NBANKS = 4                   # PSUM banks per token-tile group (4+4 ping-pong)
NG = TT // NBANKS            # 8 token-tile groups
GCOLS = P * NBANKS           # 512 tokens per group

F32 = mybir.dt.float32
BF16 = mybir.dt.bfloat16

EPS = np.float32(1e-8)
HALF_EPS = float(np.float32(0.5) * EPS)          # exact
NEG_INV_N = float(-np.float32(2.0 ** -24))       # -1/(4096*4096), exact
HALF_INV_N = float(np.float32(2.0 ** -25))


def _build_program():
    """Build and compile the per-core Bass program (identical on all cores)."""
    nc = bacc.Bacc("TRN2", target_bir_lowering=False, debug=False,
                   num_devices=N_CORES)

    xt = nc.dram_tensor("xt", [D_IN, TOK], BF16, kind="ExternalInput")
    wt = nc.dram_tensor("wt", [D_IN, O_SHARD], F32, kind="ExternalInput")
    out = nc.dram_tensor("out", [TOK, O_SHARD], F32, kind="ExternalOutput")

    rg = [list(range(N_CORES))]

    with tile.TileContext(nc) as tc:
        with (
            tc.tile_pool(name="const", bufs=1) as const,
            tc.tile_pool(name="wf", bufs=1) as wf,
            tc.tile_pool(name="wq", bufs=1) as wqp,
            tc.tile_pool(name="small", bufs=1) as small,
            tc.tile_pool(name="qtmp", bufs=4) as qtmp,
            tc.tile_pool(name="xp", bufs=8) as xp,
            tc.tile_pool(name="op", bufs=4) as op,
            tc.tile_pool(name="ps", bufs=8, space="PSUM") as ps,
            tc.tile_pool(name="dram", bufs=1, space="DRAM") as dram,
        ):
            ones_sb = const.tile([P, P], F32)
            nc.vector.memset(ones_sb[:], 1.0)

            # ---- phase 1: local sum(|w|) over the shard --------------------
            wt_sb = wf.tile([P, KT, O_SHARD], F32)       # resident fp32 shard
            partials = small.tile([P, KT], F32)
            w_dmas = []
            for k in range(KT):
                w_dmas.append(
                    nc.sync.dma_start(wt_sb[:, k, :], wt[k * P:(k + 1) * P, :]))
                nc.vector.tensor_reduce(
                    partials[:, k:k + 1], wt_sb[:, k, :],
                    axis=mybir.AxisListType.X, op=mybir.AluOpType.add,
                    apply_absolute_value=True,
                )
            partial1 = small.tile([P, 1], F32)
            nc.vector.tensor_reduce(
                partial1[:, 0:1], partials[:, :],
                axis=mybir.AxisListType.X, op=mybir.AluOpType.add,
            )

            # ---- AllGather the per-partition partials across the 8 cores ---
            # (AllGather's latency floor on 8 cores is ~half an AllReduce's;
            # the 8-way sum is folded into the ones-matmul below.)
            ag_in = dram.tile([P, 1], F32)
            ag_out = dram.tile([N_CORES * P, 1], F32)   # rank-major on dim 0
            nc.sync.dma_start(ag_in[:], partial1[:, 0:1])
            nc.gpsimd.collective_compute(
                "AllGather", mybir.AluOpType.bypass, replica_groups=rg,
                ins=[ag_in.opt()], outs=[ag_out.opt()],
            )
            # partition p <- the 8 ranks' values for partition p: [128, 8]
            gpart = small.tile([P, N_CORES], F32)
            nc.sync.dma_start(
                gpart[:, :], ag_out.opt().rearrange("(r p) c -> p (r c)", p=P))

            # reduce 8 ranks -> per-partition totals
            gpart1 = small.tile([P, 1], F32)
            nc.vector.tensor_reduce(
                gpart1[:, 0:1], gpart[:, :],
                axis=mybir.AxisListType.X, op=mybir.AluOpType.add)

            # reduce across partitions AND broadcast: ones[128,128].T @ gpart1
            psB = ps.tile([P, 512], F32, tag="acc", name="ps_bcast")
            nc.tensor.matmul(psB[:, 0:1], ones_sb[:, :], gpart1[:, 0:1],
                             start=True, stop=True)

            nscale_sb = small.tile([P, 1], F32)
            thresh_sb = small.tile([P, 1], F32)
            nthresh_sb = small.tile([P, 1], F32)
            # -scale = total * -2^-24 (exact); thresh = 0.5*(scale+eps)
            # computed as total*2^-25 + eps/2, bit-identical to the reference
            # (power-of-2 scaling commutes with fp32 rounding).
            nc.vector.tensor_scalar_mul(nscale_sb[:, 0:1], psB[:, 0:1], NEG_INV_N)
            nc.vector.tensor_scalar(
                thresh_sb[:, 0:1], psB[:, 0:1], HALF_INV_N, HALF_EPS,
                mybir.AluOpType.mult, mybir.AluOpType.add,
            )
            nc.vector.tensor_scalar_mul(nthresh_sb[:, 0:1], thresh_sb[:, 0:1], -1.0)

            # ---- ternary quantize shard -> bf16 NEGATED {-1, 0, +1} --------
            # wq = (w < -thresh) - (w > thresh) = -ternary(w); undone by -scale.
            wq_sb = wqp.tile([P, KT, O_SHARD], BF16)     # resident ternary shard
            for k in range(KT):
                pos = qtmp.tile([P, O_SHARD], BF16, tag="pos", name=f"pos_{k}")
                nc.vector.tensor_scalar(
                    pos[:], wt_sb[:, k, :], thresh_sb[:, 0:1], None,
                    mybir.AluOpType.is_gt,
                )
                nc.vector.scalar_tensor_tensor(
                    wq_sb[:, k, :], wt_sb[:, k, :], nthresh_sb[:, 0:1], pos[:],
                    mybir.AluOpType.is_lt, mybir.AluOpType.subtract,
                )

            # ---- main matmul: out[t, o] = sum_k xt[k, t] * wq[k, o] --------
            first_group = True
            for g in range(NG):
                psums = [ps.tile([P, 512], F32, tag="acc", name=f"acc_{g}_{t}")
                         for t in range(NBANKS)]
                for k in range(KT):
                    xt_t = xp.tile([P, GCOLS], BF16, tag="xt", name=f"xt_{g}_{k}")
                    xd = nc.sync.dma_start(
                        xt_t[:],
                        xt[k * P:(k + 1) * P, g * GCOLS:(g + 1) * GCOLS],
                    )
                    if first_group and k < 10:
                        # keep the first x prefetches out of the weight-shard
                        # DMA's way: the scale AllReduce (and everything after
                        # it) is serialized behind the w DMAs, while x has
                        # ~50us of slack before the matmuls start.
                        add_dep_helper(xd.ins, w_dmas[-1].ins, True,
                                       "prioritize w shard DMA")
                    for t in range(NBANKS):
                        nc.tensor.matmul(
                            psums[t][:, :O_SHARD],
                            xt_t[:, t * P:(t + 1) * P],
                            wq_sb[:, k, :],
                            start=(k == 0), stop=(k == KT - 1),
                        )
                first_group = False
                for t in range(NBANKS):
                    ot = op.tile([P, O_SHARD], F32, tag="ot", name=f"ot_{g}_{t}")
                    nc.vector.tensor_scalar_mul(
                        ot[:], psums[t][:, :O_SHARD], nscale_sb[:, 0:1])
                    row = (g * NBANKS + t) * P
                    nc.sync.dma_start(out[row:row + P, :], ot[:])

    nc.compile()
    return nc


_NC_CACHE = None


def _get_program():
    global _NC_CACHE
    if _NC_CACHE is None:
        _NC_CACHE = _build_program()
    return _NC_CACHE


def _make_in_maps(input: np.ndarray, weight: np.ndarray):
    x2d = np.ascontiguousarray(input.reshape(TOK, D_IN))
    xt_np = np.ascontiguousarray(x2d.T).astype(ml_dtypes.bfloat16)
    wT = np.ascontiguousarray(weight.T)          # [d_in, d_out] fp32
    in_maps = []
    for c in range(N_CORES):
        in_maps.append({
            "xt": xt_np,
            "wt": np.ascontiguousarray(wT[:, c * O_SHARD:(c + 1) * O_SHARD]),
        })
    return in_maps


def run_device(input: np.ndarray, weight: np.ndarray, **spmd_kwargs):
    """Run the sharded kernel; returns (full_output, BassKernelResults)."""
    nc = _get_program()
    in_maps = _make_in_maps(input, weight)
    res = run_bass_kernel_spmd(nc, in_maps, list(range(N_CORES)), **spmd_kwargs)
    shards = [res.results[c]["out"] for c in range(N_CORES)]
    full = np.concatenate(shards, axis=1).reshape(B, S, D_OUT)
    return np.ascontiguousarray(full.astype(np.float32)), res


def kernel(input: np.ndarray, weight: np.ndarray) -> np.ndarray:
    out, _ = run_device(input, weight)
    return out
